# revision 1
# baseline (speedup 1.0000x reference)
"""Trainium2 Bass kernel for the BaseMemory coref scoring module.

Computes, for full inputs (M=65536 memory slots, D=768, E=20, H=64):
    score = relu(pair @ W1 + b1) @ W2 + b2, masked with ent_counter>0,
    where pair = [mem, ment, mem*ment, dist_emb, cnt_emb].

Sharding: data-parallel over the cluster dimension M across 8 NeuronCores.
Each core's shard of mem_vectors is laid out [D, MS] (contraction-major) so
the PE consumes it directly; all FLOPs and all HBM traffic stay on device.

Key folds (host side, O(D*H) + O(M) work on the small tensors only):
  - mem@W1_mem + (mem*ment)@W1_had = mem @ (W1_mem + diag(ment)@W1_had)
  - ment@W1_ment + b1 folded into the 10-row dist bucket table
  - bucket one-hots precomputed on host (O(M) int compares) and streamed
    as a [22, MS] bf16 plane; contracted on the PE against the folded
    10-row tables (masking folded into the PE accumulation, exact)
  - mem_vectors streamed as bf16: halves HBM traffic (the roofline term);
    all accumulation stays fp32 in PSUM
"""

import os
import numpy as np

# The bass kernel executes through the axon PJRT backend; make sure jax can
# see it even if the caller pinned JAX_PLATFORMS (e.g. to "cpu").
_jp = os.environ.get("JAX_PLATFORMS")
if _jp is not None and _jp != "" and "axon" not in _jp:
    os.environ["JAX_PLATFORMS"] = "axon," + _jp

M, D, E, H = 65536, 768, 20, 64
N_CORES = 8
MS = M // N_CORES          # rows per core = 8192
GROUP = 512                # rows per PE matmul group
N_GROUPS = MS // GROUP     # 16
SG = 4                     # groups per DMA super-group
N_SG = N_GROUPS // SG      # 4
KCH = D // 128             # 6 contraction chunks
NF = 22                    # 10 dist onehot, 10 cnt onehot, notmask, ones
HT = H + NF                # 86 rows of the score-matmul rhs
BIG = float(2 ** 14)       # pre-relu kill value for masked rows (fp16-exact)

_CACHE = {}


def _build():
    """Build + compile the 8-core SPMD bass program once per process."""
    if "nc" in _CACHE:
        return _CACHE["nc"]

    import concourse.bass as bass
    import concourse.mybir as mybir
    import concourse.tile as tile
    from concourse import bacc

    F32 = mybir.dt.float32
    BF16 = mybir.dt.bfloat16
    FP16 = mybir.dt.float16

    nc = bacc.Bacc("TRN2", target_bir_lowering=False, debug=False,
                   enable_asserts=False, num_devices=N_CORES)

    NPAIR = N_GROUPS // 2      # 8 column-pair blocks per core
    PB = 2 * GROUP             # 1024 columns per pair block
    # x pre-tiled on host as [pair, partition, kchunk, col]: each DMA moves
    # one pair block with a single contiguous 12KB line per partition
    xt_d = nc.dram_tensor("xt", [NPAIR, 128, KCH, PB], BF16,
                          kind="ExternalInput").ap()
    oh_d = nc.dram_tensor("oh", [NF, MS], FP16, kind="ExternalInput").ap()
    w1_d = nc.dram_tensor("w1", [D, H], BF16, kind="ExternalInput").ap()
    tcat_d = nc.dram_tensor("tcat", [NF, H], FP16, kind="ExternalInput").ap()
    wsc_d = nc.dram_tensor("wsc", [HT, 1], FP16, kind="ExternalInput").ap()
    out_d = nc.dram_tensor("out", [MS], F32, kind="ExternalOutput").ap()

    w1_r = w1_d.rearrange("(k p) n -> p k n", p=128)    # [128, 6, 64]
    out_r = out_d.rearrange("(s c) -> s c", s=N_SG)     # [4, 2048]
    oh_r = oh_d.rearrange("f (q c) -> f q c", q=NPAIR)  # [22, 8, 1024]

    relu = mybir.ActivationFunctionType.Relu

    with tile.TileContext(nc) as tc:
        with (
            tc.tile_pool(name="consts", bufs=1) as cpool,
            tc.tile_pool(name="xin", bufs=8) as px,
            tc.tile_pool(name="hts", bufs=8) as php,
            tc.tile_pool(name="osb", bufs=2) as posb,
            tc.tile_pool(name="psz", bufs=4, space="PSUM") as psz,
            tc.tile_pool(name="pss", bufs=4, space="PSUM") as pss,
        ):
            # consts issue on the scalar HWDGE queue so the big xt DMAs
            # (sync queue) start immediately
            w1t = cpool.tile([128, KCH, H], BF16, tag="w1t")
            nc.scalar.dma_start(w1t[:], w1_r[:])
            # tcat lives at base partition 64 so lhsT/rhs base partitions
            # match in the feature-accumulation matmul
            tcat_full = cpool.tile([HT, H], FP16, tag="tcat")
            tcat = tcat_full[H:HT, :]
            nc.scalar.dma_start(tcat, tcat_d[:])
            wsc = cpool.tile([HT, 1], FP16, tag="wsc")
            nc.scalar.dma_start(wsc[:], wsc_d[:])

            osb_tiles = {}
            from collections import deque
            pending = deque()

            def emit_score(g, htp, hoff, flush=False):
                # per-group score into its own 1-bank PSUM tile; 4-deep
                # pool so reuse never waits on a recent copy (WAR slack)
                sc = pss.tile([1, GROUP], F32, tag="pss")
                nc.tensor.matmul(sc[:], wsc[:], htp[:, hoff:hoff + GROUP],
                                 start=True, stop=True,
                                 skip_group_check=True)
                sq, j = divmod(g, SG)
                if j == 0:
                    osb_new = posb.tile([1, SG * GROUP], F32, tag="osb")
                    osb_tiles[sq] = osb_new
                orow = osb_tiles[sq][0:1, GROUP * j:GROUP * (j + 1)]
                if g % 2 == 0:
                    nc.scalar.copy(orow, sc[:])
                else:
                    nc.vector.tensor_copy(orow, sc[:])
                if j == SG - 1:
                    # the last super-group output rides the by-then idle
                    # sync HWDGE ring (lower fixed latency than SWDGE)
                    eng = nc.sync if sq == N_SG - 1 else nc.gpsimd
                    eng.dma_start(out_r[sq:sq + 1, :], osb_tiles.pop(sq)[:])

            # all pair-block loads issue upfront on the sync ring: DMA
            # streams at full rate in consumption order, one fat
            # contiguous descriptor per partition
            xqs = []
            for q in range(NPAIR):
                xq = px.tile([128, KCH, PB], BF16, tag="xin")
                if q == NPAIR - 1:
                    # last block lands in halves so only the final group's
                    # matmuls trail the last byte
                    nc.sync.dma_start(xq[:, :, 0:GROUP],
                                      xt_d[q][:, :, 0:GROUP])
                    nc.sync.dma_start(xq[:, :, GROUP:PB],
                                      xt_d[q][:, :, GROUP:PB])
                else:
                    nc.sync.dma_start(xq[:], xt_d[q])
                xqs.append(xq)

            # per-pair score-rhs tiles: rows 0..63 relu(z), rows 64..85 the
            # host-computed one-hot plane (own buffer per pair, so no
            # write-after-read false deps); all 8 loads issue upfront on
            # the scalar ring right after the consts
            htps = []
            for q in range(NPAIR):
                htp = php.tile([HT, PB], FP16, tag="hts")
                nc.scalar.dma_start(htp[H:HT, :], oh_r[:, q, :])
                htps.append(htp)

            for q in range(NPAIR):
                htp = htps[q]
                xq = xqs[q]
                # two groups per PSUM tile, computed on the two PE column
                # halves concurrently (tile_position inferred from the
                # output base partition)
                zt = psz.tile([2 * H, GROUP], F32, tag="psz")
                if q == NPAIR - 1:
                    # separate PSUM tile for the last pair's B group, so
                    # relu-A's (tile-granular) dependency does not include
                    # the B matmuls that wait on the final half-block DMA
                    ztb = psz.tile([2 * H, GROUP], F32, tag="psz")
                else:
                    ztb = zt
                # feature/bias/mask contribution first: it depends only on
                # the (early) one-hot DMA, so it fills PE idle time while
                # x streams in, and relu fires right after k==5
                nc.tensor.matmul(zt[0:H, :], tcat, htp[H:HT, 0:GROUP],
                                 start=True, stop=False,
                                 skip_group_check=True)
                nc.tensor.matmul(ztb[H:2 * H, :], tcat,
                                 htp[H:HT, GROUP:PB],
                                 start=True, stop=False,
                                 skip_group_check=True)
                if q == NPAIR - 1:
                    # last pair: finish the A group on the first half-block
                    # DMA before any B matmul can block the PE FIFO on the
                    # second half-block
                    for k in range(KCH):
                        nc.tensor.matmul(zt[0:H, :], w1t[:, k, :],
                                         xq[:, k, 0:GROUP],
                                         start=False, stop=(k == KCH - 1),
                                         skip_group_check=True)
                    for k in range(KCH):
                        nc.tensor.matmul(ztb[H:2 * H, :], w1t[:, k, :],
                                         xq[:, k, GROUP:PB],
                                         start=False, stop=(k == KCH - 1),
                                         skip_group_check=True)
                else:
                    for k in range(KCH):
                        nc.tensor.matmul(zt[0:H, :], w1t[:, k, :],
                                         xq[:, k, 0:GROUP],
                                         start=False, stop=(k == KCH - 1),
                                         skip_group_check=True)
                        nc.tensor.matmul(ztb[H:2 * H, :], w1t[:, k, :],
                                         xq[:, k, GROUP:PB],
                                         start=False, stop=(k == KCH - 1),
                                         skip_group_check=True)

                # relus run concurrently on ACT and DVE
                nc.scalar.activation(htp[0:H, 0:GROUP], zt[0:H, :], relu)
                nc.vector.tensor_scalar_max(htp[0:H, GROUP:PB],
                                            ztb[H:2 * H, :], 0.0)
                pending.append((2 * q, htp, 0))
                pending.append((2 * q + 1, htp, GROUP))
                # scores trail by two pairs so they never stall the PE
                # queue waiting on a relu
                while len(pending) >= 5:
                    emit_score(*pending.popleft())
            while pending:
                emit_score(*pending.popleft(), flush=True)

    nc.compile()
    _CACHE["nc"] = nc
    return nc


def _bucket(c):
    """Reference get_bucket, replicated with the same XLA CPU float ops so
    boundary cases (c = 8, 16, 32) bucket identically."""
    import math
    import jax
    import jax.numpy as jnp
    cpu = jax.devices("cpu")[0]
    with jax.default_device(cpu):
        c = jnp.asarray(c).astype(jnp.int32)
        logspace = jnp.floor(
            jnp.log(jnp.maximum(c, 1).astype(jnp.float32)) / math.log(2)
        ).astype(jnp.int32) + 3
        idx = jnp.where(c <= 4, c, logspace)
        return np.asarray(jnp.clip(idx, 0, 9))


def _prepare_maps(ment_emb, mem_vectors, dist_table, counter_table,
                  W1, b1, W2, b2, ent_counter, last_mention_start, ment_start):
    import ml_dtypes
    f32 = np.float32
    bf16 = ml_dtypes.bfloat16
    fp16 = np.float16
    ment = np.asarray(ment_emb, f32)
    mem = np.asarray(mem_vectors, f32)
    W1 = np.asarray(W1, f32)
    ms = int(np.asarray(ment_start))

    W1m, W1r, W1h = W1[0:D], W1[D:2 * D], W1[2 * D:3 * D]
    W1d, W1c = W1[3 * D:3 * D + E], W1[3 * D + E:3 * D + 2 * E]

    w1eff = (W1m + ment[:, None] * W1h).astype(f32)              # [768, 64]
    bias_vec = (np.asarray(b1, f32) + ment @ W1r).astype(f32)    # [64]
    T_d = (np.asarray(dist_table, f32) @ W1d + bias_vec).astype(f32)
    T_c = (np.asarray(counter_table, f32) @ W1c).astype(f32)
    b2v = float(np.asarray(b2, f32).reshape(-1)[0])

    tcat = np.concatenate(
        [T_d, T_c, np.full((1, H), -BIG, f32), np.zeros((1, H), f32)], 0)
    # single score matmul: rows 0..63 act on relu(z), rows 64..85 on onehot
    wsc = np.zeros((HT, 1), f32)
    wsc[0:H, 0] = np.asarray(W2, f32).reshape(-1)
    wsc[H + 20, 0] = -10000.0 - b2v
    wsc[H + 21, 0] = b2v

    cnt_i = np.asarray(ent_counter).astype(np.int64)
    dist_i = ms - np.asarray(last_mention_start).astype(np.int64)
    bd = _bucket(dist_i)                                         # [M] in 0..9
    bc = _bucket(cnt_i)                                          # [M] in 0..9
    r = np.arange(10)
    oh = np.empty((NF, M), f32)
    oh[0:10] = (bd[None, :] == r[:, None])
    oh[10:20] = (bc[None, :] == r[:, None])
    oh[20] = (cnt_i <= 0)
    oh[21] = 1.0
    oh = oh.astype(fp16)

    w1_b = w1eff.astype(bf16)
    tcat_b = tcat.astype(fp16)
    wsc_b = wsc.astype(fp16)

    in_maps = []
    for c in range(N_CORES):
        sl = slice(c * MS, (c + 1) * MS)
        # [pair, partition, kchunk, col] pre-tiling: one contiguous 12KB
        # line per partition per pair-block DMA
        xt = (mem[sl].T.astype(bf16)
              .reshape(KCH, 128, MS // 1024, 1024)
              .transpose(2, 1, 0, 3))
        in_maps.append(dict(
            xt=np.ascontiguousarray(xt),
            oh=np.ascontiguousarray(oh[:, sl]),
            w1=w1_b, tcat=tcat_b, wsc=wsc_b))
    return in_maps


def _postprocess(results):
    out = np.empty(M + 1, np.float32)
    for c in range(N_CORES):
        out[c * MS:(c + 1) * MS] = results[c]["out"]
    out[M] = 0.0
    return out


def run_spmd(in_maps, trace=False):
    from concourse.bass_utils import run_bass_kernel_spmd
    nc = _build()
    return run_bass_kernel_spmd(nc, in_maps, list(range(N_CORES)), trace=trace)


def kernel(**inputs):
    in_maps = _prepare_maps(**inputs)
    res = run_spmd(in_maps, trace=False)
    return _postprocess(res.results)



# revision 2
# speedup vs baseline: 1.0074x; 1.0074x over previous
"""Trainium2 Bass kernel for the BaseMemory coref scoring module.

Computes, for full inputs (M=65536 memory slots, D=768, E=20, H=64):
    score = relu(pair @ W1 + b1) @ W2 + b2, masked with ent_counter>0,
    where pair = [mem, ment, mem*ment, dist_emb, cnt_emb].

Sharding: data-parallel over the cluster dimension M across 8 NeuronCores.
Each core's shard of mem_vectors is laid out [D, MS] (contraction-major) so
the PE consumes it directly; all FLOPs and all HBM traffic stay on device.

Key folds (host side, O(D*H) + O(M) work on the small tensors only):
  - mem@W1_mem + (mem*ment)@W1_had = mem @ (W1_mem + diag(ment)@W1_had)
  - ment@W1_ment + b1 folded into the 10-row dist bucket table
  - bucket one-hots precomputed on host (O(M) int compares) and streamed
    as ONE [22, MS] fp16 plane; contracted on the PE against the folded
    10-row tables (masking folded into the PE accumulation, exact)
  - mem_vectors streamed as bf16: halves HBM traffic (the roofline term);
    all accumulation stays fp32 in PSUM

Scheduling (v2): the xt stream is split across BOTH HWDGE rings (sync +
scalar) and the small loads ride the gpsimd SWDGE queue, so only ~5 DMA
issues sit on each ring and the 8 DMA-completion sem lanes recycle against
early-completing transfers.  The score matmul is split into two
accumulating matmuls (W2 x relu(z) + wsc_oh x onehot) so no per-pair
one-hot staging tiles exist.  Each pair's z accumulation starts at k=0
(the bucket-table matmul closes the chain) so the PE starts as soon as
the first x bytes land and stays dense enough to hold the HAM clock at
2.4 GHz.
"""

import os
import numpy as np

# The bass kernel executes through the axon PJRT backend; make sure jax can
# see it even if the caller pinned JAX_PLATFORMS (e.g. to "cpu").
_jp = os.environ.get("JAX_PLATFORMS")
if _jp is not None and _jp != "" and "axon" not in _jp:
    os.environ["JAX_PLATFORMS"] = "axon," + _jp

M, D, E, H = 65536, 768, 20, 64
N_CORES = 8
MS = M // N_CORES          # rows per core = 8192
GROUP = 512                # rows per PE matmul group
N_GROUPS = MS // GROUP     # 16
SG = 4                     # groups per output super-group
N_SG = N_GROUPS // SG      # 4
KCH = D // 128             # 6 contraction chunks
NF = 22                    # 10 dist onehot, 10 cnt onehot, notmask, ones
NPAIR = N_GROUPS // 2      # 8 column-pair blocks per core
PB = 2 * GROUP             # 1024 columns per pair block
BIG = float(2 ** 14)       # pre-relu kill value for masked rows (fp16-exact)

_CACHE = {}


def _build():
    """Build + compile the 8-core SPMD bass program once per process."""
    if "nc" in _CACHE:
        return _CACHE["nc"]

    import concourse.bass as bass
    import concourse.mybir as mybir
    import concourse.tile as tile
    from concourse import bacc

    F32 = mybir.dt.float32
    BF16 = mybir.dt.bfloat16
    FP16 = mybir.dt.float16

    nc = bacc.Bacc("TRN2", target_bir_lowering=False, debug=False,
                   enable_asserts=False, num_devices=N_CORES)

    # x pre-tiled on host as [pair, partition, kchunk, col]: each DMA moves
    # one pair block with a single contiguous 12KB line per partition
    xt_d = nc.dram_tensor("xt", [NPAIR, 128, KCH, PB], BF16,
                          kind="ExternalInput").ap()
    oh_d = nc.dram_tensor("oh", [NF, MS], FP16, kind="ExternalInput").ap()
    w1_d = nc.dram_tensor("w1", [128, KCH * H], BF16,
                          kind="ExternalInput").ap()
    # packed small consts: cols 0..63 rows 0..21 = folded bucket tables,
    # col 64 = W2, col 65 rows 20/21 = mask/bias score weights
    p_d = nc.dram_tensor("pk", [H, 66], FP16, kind="ExternalInput").ap()
    out_d = nc.dram_tensor("out", [MS], F32, kind="ExternalOutput").ap()

    w1_r = w1_d.rearrange("p (k n) -> p k n", k=KCH)    # [128, 6, 64]
    out_r = out_d.rearrange("(s c) -> s c", s=N_SG)     # [4, 2048]

    relu = mybir.ActivationFunctionType.Relu

    with tile.TileContext(nc) as tc:
        with (
            tc.tile_pool(name="consts", bufs=1) as cpool,
            tc.tile_pool(name="xin", bufs=8) as px,
            tc.tile_pool(name="hts", bufs=8) as ph,
            tc.tile_pool(name="osb", bufs=2) as posb,
            tc.tile_pool(name="psz", bufs=4, space="PSUM") as psz,
            tc.tile_pool(name="pss", bufs=4, space="PSUM") as pss,
        ):
            # small consts ride the gpsimd SWDGE queue: their issue cost
            # overlaps the HWDGE ring issues, and they complete early so
            # the 8 DMA-completion lanes recycle against them
            w1t = cpool.tile([128, KCH, H], BF16, tag="w1t")
            nc.gpsimd.dma_start(w1t[:], w1_r[:])
            pk = cpool.tile([H, 66], FP16, tag="pk")
            nc.gpsimd.dma_start(pk[:], p_d[:])
            ohb = cpool.tile([NF, MS], FP16, tag="ohb")
            nc.gpsimd.dma_start(ohb[:], oh_d[:])

            # pair-block loads split across BOTH HWDGE rings in consumption
            # order; first and last blocks land in halves so the PE starts
            # ~2us earlier and only the final group trails the last byte
            xqs = []
            for q in range(NPAIR):
                xq = px.tile([128, KCH, PB], BF16, tag="xin")
                eng = nc.sync if q % 2 == 0 else nc.scalar
                if q == 0 or q == NPAIR - 1:
                    eng.dma_start(xq[:, :, 0:GROUP], xt_d[q][:, :, 0:GROUP])
                    eng.dma_start(xq[:, :, GROUP:PB], xt_d[q][:, :, GROUP:PB])
                else:
                    eng.dma_start(xq[:], xt_d[q])
                xqs.append(xq)

            osb_tiles = {}
            from collections import deque
            pending = deque()

            def emit_score(g, hq, hoff):
                # per-group score: two accumulating matmuls into one
                # 1-bank PSUM tile (W2 x relu(z), then wsc_oh x onehot)
                sc = pss.tile([1, GROUP], F32, tag="pss")
                nc.tensor.matmul(sc[:], pk[0:H, 64:65],
                                 hq[0:H, hoff:hoff + GROUP],
                                 start=True, stop=False,
                                 skip_group_check=True)
                nc.tensor.matmul(sc[:], pk[0:NF, 65:66],
                                 ohb[:, GROUP * g:GROUP * (g + 1)],
                                 start=False, stop=True,
                                 skip_group_check=True)
                sq, j = divmod(g, SG)
                if j == 0:
                    osb_new = posb.tile([1, SG * GROUP], F32, tag="osb")
                    osb_tiles[sq] = osb_new
                orow = osb_tiles[sq][0:1, GROUP * j:GROUP * (j + 1)]
                if g % 2 == 0:
                    nc.scalar.copy(orow, sc[:])
                else:
                    nc.vector.tensor_copy(orow, sc[:])
                if j == SG - 1:
                    # the last super-group output rides the by-then idle
                    # sync HWDGE ring (lower fixed latency than SWDGE)
                    eng = nc.sync if sq == N_SG - 1 else nc.gpsimd
                    eng.dma_start(out_r[sq:sq + 1, :], osb_tiles.pop(sq)[:])

            for q in range(NPAIR):
                xq = xqs[q]
                # two groups per PSUM tile, computed on the two PE column
                # halves concurrently (tile_position inferred from the
                # output base partition)
                zt = psz.tile([2 * H, GROUP], F32, tag="psz")
                if q == NPAIR - 1:
                    # separate PSUM tile for the last pair's B group, so
                    # relu-A's (tile-granular) dependency does not include
                    # the B matmuls that wait on the final half-block DMA
                    ztb = psz.tile([2 * H, GROUP], F32, tag="psz")
                else:
                    ztb = zt
                # z chains open on k=0 so the PE starts on the first x
                # bytes; the bucket-table matmul (deps: ohb+pk only)
                # closes each chain
                if q == NPAIR - 1:
                    for k in range(KCH):
                        nc.tensor.matmul(zt[0:H, :], w1t[:, k, :],
                                         xq[:, k, 0:GROUP],
                                         start=(k == 0), stop=False,
                                         skip_group_check=True)
                    for k in range(KCH):
                        nc.tensor.matmul(ztb[H:2 * H, :], w1t[:, k, :],
                                         xq[:, k, GROUP:PB],
                                         start=(k == 0), stop=False,
                                         skip_group_check=True)
                else:
                    for k in range(KCH):
                        nc.tensor.matmul(zt[0:H, :], w1t[:, k, :],
                                         xq[:, k, 0:GROUP],
                                         start=(k == 0), stop=False,
                                         skip_group_check=True)
                        nc.tensor.matmul(ztb[H:2 * H, :], w1t[:, k, :],
                                         xq[:, k, GROUP:PB],
                                         start=(k == 0), stop=False,
                                         skip_group_check=True)
                nc.tensor.matmul(zt[0:H, :], pk[0:NF, 0:H],
                                 ohb[:, PB * q:PB * q + GROUP],
                                 start=False, stop=True,
                                 skip_group_check=True)
                nc.tensor.matmul(ztb[H:2 * H, :], pk[0:NF, 0:H],
                                 ohb[:, PB * q + GROUP:PB * (q + 1)],
                                 start=False, stop=True,
                                 skip_group_check=True)

                # relus run concurrently on ACT and DVE
                hq = ph.tile([H, PB], FP16, tag="hts")
                nc.scalar.activation(hq[:, 0:GROUP], zt[0:H, :], relu)
                nc.vector.tensor_scalar_max(hq[:, GROUP:PB],
                                            ztb[H:2 * H, :], 0.0)
                pending.append((2 * q, hq, 0))
                pending.append((2 * q + 1, hq, GROUP))
                # scores trail by two pairs so they never stall the PE
                # queue waiting on a relu
                while len(pending) >= 5:
                    emit_score(*pending.popleft())
            while pending:
                emit_score(*pending.popleft())

    nc.compile()
    _CACHE["nc"] = nc
    return nc


def _bucket(c):
    """Reference get_bucket, replicated with the same XLA CPU float ops so
    boundary cases (c = 8, 16, 32) bucket identically."""
    import math
    import jax
    import jax.numpy as jnp
    cpu = jax.devices("cpu")[0]
    with jax.default_device(cpu):
        c = jnp.asarray(c).astype(jnp.int32)
        logspace = jnp.floor(
            jnp.log(jnp.maximum(c, 1).astype(jnp.float32)) / math.log(2)
        ).astype(jnp.int32) + 3
        idx = jnp.where(c <= 4, c, logspace)
        return np.asarray(jnp.clip(idx, 0, 9))


def _prepare_maps(ment_emb, mem_vectors, dist_table, counter_table,
                  W1, b1, W2, b2, ent_counter, last_mention_start, ment_start):
    import ml_dtypes
    f32 = np.float32
    bf16 = ml_dtypes.bfloat16
    fp16 = np.float16
    ment = np.asarray(ment_emb, f32)
    mem = np.asarray(mem_vectors, f32)
    W1 = np.asarray(W1, f32)
    ms = int(np.asarray(ment_start))

    W1m, W1r, W1h = W1[0:D], W1[D:2 * D], W1[2 * D:3 * D]
    W1d, W1c = W1[3 * D:3 * D + E], W1[3 * D + E:3 * D + 2 * E]

    w1eff = (W1m + ment[:, None] * W1h).astype(f32)              # [768, 64]
    bias_vec = (np.asarray(b1, f32) + ment @ W1r).astype(f32)    # [64]
    T_d = (np.asarray(dist_table, f32) @ W1d + bias_vec).astype(f32)
    T_c = (np.asarray(counter_table, f32) @ W1c).astype(f32)
    b2v = float(np.asarray(b2, f32).reshape(-1)[0])

    # pk: cols 0..63 = folded bucket tables (rows 0..9 dist incl. bias,
    # 10..19 counter, 20 = -BIG mask kill, 21 unused); col 64 = W2;
    # col 65 = onehot-side score weights (mask -10000 and bias b2)
    pk = np.zeros((H, 66), f32)
    pk[0:10, 0:H] = T_d
    pk[10:20, 0:H] = T_c
    pk[20, 0:H] = -BIG
    pk[0:H, 64] = np.asarray(W2, f32).reshape(-1)
    pk[20, 65] = -10000.0 - b2v
    pk[21, 65] = b2v

    cnt_i = np.asarray(ent_counter).astype(np.int64)
    dist_i = ms - np.asarray(last_mention_start).astype(np.int64)
    bd = _bucket(dist_i)                                         # [M] in 0..9
    bc = _bucket(cnt_i)                                          # [M] in 0..9
    r = np.arange(10)
    oh = np.empty((NF, M), f32)
    oh[0:10] = (bd[None, :] == r[:, None])
    oh[10:20] = (bc[None, :] == r[:, None])
    oh[20] = (cnt_i <= 0)
    oh[21] = 1.0
    oh = oh.astype(fp16)

    # w1 laid out [128, kchunk*64] so the load is one contiguous 768B
    # line per partition
    w1_b = np.ascontiguousarray(
        w1eff.reshape(KCH, 128, H).transpose(1, 0, 2).reshape(128, KCH * H)
    ).astype(bf16)
    pk_b = pk.astype(fp16)

    in_maps = []
    for c in range(N_CORES):
        sl = slice(c * MS, (c + 1) * MS)
        # [pair, partition, kchunk, col] pre-tiling: one contiguous 12KB
        # line per partition per pair-block DMA
        xt = (mem[sl].T.astype(bf16)
              .reshape(KCH, 128, MS // 1024, 1024)
              .transpose(2, 1, 0, 3))
        in_maps.append(dict(
            xt=np.ascontiguousarray(xt),
            oh=np.ascontiguousarray(oh[:, sl]),
            w1=w1_b, pk=pk_b))
    return in_maps


def _postprocess(results):
    out = np.empty(M + 1, np.float32)
    for c in range(N_CORES):
        out[c * MS:(c + 1) * MS] = results[c]["out"]
    out[M] = 0.0
    return out


def run_spmd(in_maps, trace=False):
    from concourse.bass_utils import run_bass_kernel_spmd
    nc = _build()
    return run_bass_kernel_spmd(nc, in_maps, list(range(N_CORES)), trace=trace)


def kernel(**inputs):
    in_maps = _prepare_maps(**inputs)
    res = run_spmd(in_maps, trace=False)
    return _postprocess(res.results)


# revision 4
# speedup vs baseline: 1.1040x; 1.0960x over previous
"""Trainium2 Bass kernel for the BaseMemory coref scoring module.

Computes, for full inputs (M=65536 memory slots, D=768, E=20, H=64):
    score = relu(pair @ W1 + b1) @ W2 + b2, masked with ent_counter>0,
    where pair = [mem, ment, mem*ment, dist_emb, cnt_emb].

Sharding: data-parallel over the cluster dimension M across 8 NeuronCores.
Each core's shard of mem_vectors is laid out [D, MS] (contraction-major) so
the PE consumes it directly; all FLOPs and all HBM traffic stay on device.

Key folds (host side, O(D*H) + O(M) work on the small tensors only):
  - mem@W1_mem + (mem*ment)@W1_had = mem @ (W1_mem + diag(ment)@W1_had)
  - ment@W1_ment + b1 folded into the 10-row dist bucket table
  - bucket one-hots precomputed on host (O(M) int compares) and streamed
    as ONE [22, MS] fp16 plane; contracted on the PE against the folded
    10-row tables (masking folded into the PE accumulation, exact)
  - mem_vectors streamed as bf16: halves HBM traffic (the roofline term);
    all accumulation stays fp32 in PSUM

Scheduling (v3), from trace analysis:
  - SDMA bandwidth share between queues is ~proportional to descriptor
    size, so every x transfer is a contiguous half-block with uniform
    6KB-per-partition lines: A halves on the sync HWDGE ring, B halves
    on the scalar ring -> fair 50/50 split in consumption order.
  - DMA issue #n blocks until DMA #(n-8) completes (8 completion lanes),
    so the kernel keeps few, early-completing transfers in front.
  - w1 (768B lines, would be starved ~8:1) rides inside the first
    half-block's DMA as 64 extra columns per k-chunk.
  - The score matmul is split into two accumulating matmuls
    (W2 x relu(z) + wsc_oh x onehot) so no per-pair one-hot staging
    tiles or extra DMAs exist.
  - Each pair's z accumulation starts at k=0 (the bucket-table matmul
    closes the chain) so the PE starts on the first x bytes and holds
    the HAM clock at 2.4 GHz.
"""

import os
import numpy as np

# The bass kernel executes through the axon PJRT backend; make sure jax can
# see it even if the caller pinned JAX_PLATFORMS (e.g. to "cpu").
_jp = os.environ.get("JAX_PLATFORMS")
if _jp is not None and _jp != "" and "axon" not in _jp:
    os.environ["JAX_PLATFORMS"] = "axon," + _jp

M, D, E, H = 65536, 768, 20, 64
N_CORES = 8
MS = M // N_CORES          # rows per core = 8192
GROUP = 512                # rows per PE matmul group
N_GROUPS = MS // GROUP     # 16
SG = 4                     # groups per output super-group
N_SG = N_GROUPS // SG      # 4
KCH = D // 128             # 6 contraction chunks
NF = 22                    # 10 dist onehot, 10 cnt onehot, notmask, ones
NPAIR = N_GROUPS // 2      # 8 column-pair blocks per core
PB = 2 * GROUP             # 1024 columns per pair block
BIG = float(2 ** 14)       # pre-relu kill value for masked rows (fp16-exact)

_CACHE = {}


def _build():
    """Build + compile the 8-core SPMD bass program once per process."""
    if "nc" in _CACHE:
        return _CACHE["nc"]

    import concourse.bass as bass
    import concourse.mybir as mybir
    import concourse.tile as tile
    from concourse import bacc

    F32 = mybir.dt.float32
    BF16 = mybir.dt.bfloat16
    FP16 = mybir.dt.float16

    nc = bacc.Bacc("TRN2", target_bir_lowering=False, debug=False,
                   enable_asserts=False, num_devices=N_CORES)

    # x pre-tiled on host as contiguous half-blocks [hb, partition,
    # kchunk, col]: each DMA moves one half-block with a single 6KB
    # contiguous line per partition.  Half-block 0 carries w1 as 64
    # extra columns per k-chunk (so w1 is never bandwidth-starved).
    x0_d = nc.dram_tensor("x0", [128, KCH, GROUP + H], BF16,
                          kind="ExternalInput").ap()
    xt_d = nc.dram_tensor("xt", [2 * NPAIR - 1, 128, KCH, GROUP], BF16,
                          kind="ExternalInput").ap()
    oh_d = nc.dram_tensor("oh", [NF, MS], FP16, kind="ExternalInput").ap()
    # packed small consts: cols 0..63 rows 0..21 = folded bucket tables,
    # col 64 = W2, col 65 rows 20/21 = mask/bias score weights
    p_d = nc.dram_tensor("pk", [H, 66], FP16, kind="ExternalInput").ap()
    out_d = nc.dram_tensor("out", [MS], F32, kind="ExternalOutput").ap()

    out_r = out_d.rearrange("(s c) -> s c", s=N_SG)     # [4, 2048]

    relu = mybir.ActivationFunctionType.Relu

    with tile.TileContext(nc) as tc:
        with (
            tc.tile_pool(name="consts", bufs=1) as cpool,
            tc.tile_pool(name="xin", bufs=16) as px,
            tc.tile_pool(name="hts", bufs=8) as ph,
            tc.tile_pool(name="osb", bufs=2) as posb,
            tc.tile_pool(name="psz", bufs=4, space="PSUM") as psz,
            tc.tile_pool(name="pss", bufs=4, space="PSUM") as pss,
        ):
            # half-block loads in consumption order: A halves (+w1 in the
            # first) on the sync ring, oh then B halves on the scalar
            # ring; pk rides the otherwise idle gpsimd SWDGE queue
            x0 = cpool.tile([128, KCH, GROUP + H], BF16, tag="x0")
            nc.sync.dma_start(x0[:], x0_d[:])
            ohb = cpool.tile([NF, MS], FP16, tag="ohb")
            nc.scalar.dma_start(ohb[:], oh_d[:])
            pk = cpool.tile([H, 66], FP16, tag="pk")
            nc.gpsimd.dma_start(pk[:], p_d[:])

            halves = [x0]
            for hb in range(1, 2 * NPAIR):
                xh = px.tile([128, KCH, GROUP], BF16, tag="xin")
                eng = nc.sync if hb % 2 == 0 else nc.scalar
                eng.dma_start(xh[:], xt_d[hb - 1])
                halves.append(xh)

            def w1(k):
                return x0[:, k, GROUP:GROUP + H]

            osb_tiles = {}
            from collections import deque
            pending = deque()

            def emit_score(g, hq, hoff):
                # per-group score: two accumulating matmuls into one
                # 1-bank PSUM tile (W2 x relu(z), then wsc_oh x onehot)
                sc = pss.tile([1, GROUP], F32, tag="pss")
                nc.tensor.matmul(sc[:], pk[0:H, 64:65],
                                 hq[0:H, hoff:hoff + GROUP],
                                 start=True, stop=False,
                                 skip_group_check=True)
                nc.tensor.matmul(sc[:], pk[0:NF, 65:66],
                                 ohb[:, GROUP * g:GROUP * (g + 1)],
                                 start=False, stop=True,
                                 skip_group_check=True)
                sq, j = divmod(g, SG)
                if j == 0:
                    osb_new = posb.tile([1, SG * GROUP], F32, tag="osb")
                    osb_tiles[sq] = osb_new
                orow = osb_tiles[sq][0:1, GROUP * j:GROUP * (j + 1)]
                # odd groups copy on ACT so the final (odd) group's copy
                # never queues behind the final relu on DVE
                if g % 2 == 0:
                    nc.vector.tensor_copy(orow, sc[:])
                else:
                    nc.scalar.copy(orow, sc[:])
                if j == SG - 1:
                    # the last super-group output rides the by-then idle
                    # sync HWDGE ring (lower fixed latency than SWDGE)
                    eng = nc.sync if sq == N_SG - 1 else nc.gpsimd
                    eng.dma_start(out_r[sq:sq + 1, :], osb_tiles.pop(sq)[:])

            for q in range(NPAIR):
                xa = halves[2 * q]
                xb = halves[2 * q + 1]
                # two groups per PSUM tile, computed on the two PE column
                # halves concurrently (tile_position inferred from the
                # output base partition)
                zt = psz.tile([2 * H, GROUP], F32, tag="psz")
                if q == NPAIR - 1:
                    # separate PSUM tile for the last pair's B group, so
                    # relu-A's (tile-granular) dependency does not include
                    # the B matmuls that wait on the final half-block DMA
                    ztb = psz.tile([2 * H, GROUP], F32, tag="psz")
                else:
                    ztb = zt
                # z chains open on k=0 so the PE starts on the first x
                # bytes; the bucket-table matmul (deps: ohb+pk only)
                # closes each chain
                if q == NPAIR - 1:
                    for k in range(KCH):
                        nc.tensor.matmul(zt[0:H, :], w1(k), xa[:, k, 0:GROUP],
                                         start=(k == 0), stop=False,
                                         skip_group_check=True)
                    for k in range(KCH):
                        nc.tensor.matmul(ztb[H:2 * H, :], w1(k), xb[:, k, 0:GROUP],
                                         start=(k == 0), stop=False,
                                         skip_group_check=True)
                else:
                    for k in range(KCH):
                        nc.tensor.matmul(zt[0:H, :], w1(k), xa[:, k, 0:GROUP],
                                         start=(k == 0), stop=False,
                                         skip_group_check=True)
                        nc.tensor.matmul(ztb[H:2 * H, :], w1(k), xb[:, k, 0:GROUP],
                                         start=(k == 0), stop=False,
                                         skip_group_check=True)
                nc.tensor.matmul(zt[0:H, :], pk[0:NF, 0:H],
                                 ohb[:, PB * q:PB * q + GROUP],
                                 start=False, stop=True,
                                 skip_group_check=True)
                nc.tensor.matmul(ztb[H:2 * H, :], pk[0:NF, 0:H],
                                 ohb[:, PB * q + GROUP:PB * (q + 1)],
                                 start=False, stop=True,
                                 skip_group_check=True)

                # relus run concurrently on ACT and DVE
                hq = ph.tile([H, PB], FP16, tag="hts")
                nc.scalar.activation(hq[:, 0:GROUP], zt[0:H, :], relu)
                nc.vector.tensor_scalar_max(hq[:, GROUP:PB],
                                            ztb[H:2 * H, :], 0.0)
                pending.append((2 * q, hq, 0))
                pending.append((2 * q + 1, hq, GROUP))
                # scores trail by two pairs so they never stall the PE
                # queue waiting on a relu
                while len(pending) >= 5:
                    emit_score(*pending.popleft())
            while pending:
                emit_score(*pending.popleft())

    nc.compile()
    _CACHE["nc"] = nc
    return nc


def _bucket(c):
    """Reference get_bucket, replicated with the same XLA CPU float ops so
    boundary cases (c = 8, 16, 32) bucket identically."""
    import math
    import jax
    import jax.numpy as jnp
    cpu = jax.devices("cpu")[0]
    with jax.default_device(cpu):
        c = jnp.asarray(c).astype(jnp.int32)
        logspace = jnp.floor(
            jnp.log(jnp.maximum(c, 1).astype(jnp.float32)) / math.log(2)
        ).astype(jnp.int32) + 3
        idx = jnp.where(c <= 4, c, logspace)
        return np.asarray(jnp.clip(idx, 0, 9))


def _prepare_maps(ment_emb, mem_vectors, dist_table, counter_table,
                  W1, b1, W2, b2, ent_counter, last_mention_start, ment_start):
    import ml_dtypes
    f32 = np.float32
    bf16 = ml_dtypes.bfloat16
    fp16 = np.float16
    ment = np.asarray(ment_emb, f32)
    mem = np.asarray(mem_vectors, f32)
    W1 = np.asarray(W1, f32)
    ms = int(np.asarray(ment_start))

    W1m, W1r, W1h = W1[0:D], W1[D:2 * D], W1[2 * D:3 * D]
    W1d, W1c = W1[3 * D:3 * D + E], W1[3 * D + E:3 * D + 2 * E]

    w1eff = (W1m + ment[:, None] * W1h).astype(f32)              # [768, 64]
    bias_vec = (np.asarray(b1, f32) + ment @ W1r).astype(f32)    # [64]
    T_d = (np.asarray(dist_table, f32) @ W1d + bias_vec).astype(f32)
    T_c = (np.asarray(counter_table, f32) @ W1c).astype(f32)
    b2v = float(np.asarray(b2, f32).reshape(-1)[0])

    # pk: cols 0..63 = folded bucket tables (rows 0..9 dist incl. bias,
    # 10..19 counter, 20 = -BIG mask kill, 21 unused); col 64 = W2;
    # col 65 = onehot-side score weights (mask -10000 and bias b2)
    pk = np.zeros((H, 66), f32)
    pk[0:10, 0:H] = T_d
    pk[10:20, 0:H] = T_c
    pk[20, 0:H] = -BIG
    pk[0:H, 64] = np.asarray(W2, f32).reshape(-1)
    pk[20, 65] = -10000.0 - b2v
    pk[21, 65] = b2v

    cnt_i = np.asarray(ent_counter).astype(np.int64)
    dist_i = ms - np.asarray(last_mention_start).astype(np.int64)
    bd = _bucket(dist_i)                                         # [M] in 0..9
    bc = _bucket(cnt_i)                                          # [M] in 0..9
    r = np.arange(10)
    oh = np.empty((NF, M), f32)
    oh[0:10] = (bd[None, :] == r[:, None])
    oh[10:20] = (bc[None, :] == r[:, None])
    oh[20] = (cnt_i <= 0)
    oh[21] = 1.0
    oh = oh.astype(fp16)

    # w1 tiled [128, kchunk, 64] to ride inside the first half-block
    w1_t = (w1eff.reshape(KCH, 128, H).transpose(1, 0, 2)).astype(bf16)
    pk_b = pk.astype(fp16)

    in_maps = []
    for c in range(N_CORES):
        sl = slice(c * MS, (c + 1) * MS)
        # [hb, partition, kchunk, col] pre-tiling: one contiguous 6KB
        # line per partition per half-block DMA
        xt = (mem[sl].T.astype(bf16)
              .reshape(KCH, 128, 2 * NPAIR, GROUP)
              .transpose(2, 1, 0, 3))
        x0 = np.concatenate([xt[0], w1_t], axis=2)   # [128, KCH, 576]
        in_maps.append(dict(
            x0=np.ascontiguousarray(x0),
            xt=np.ascontiguousarray(xt[1:]),
            oh=np.ascontiguousarray(oh[:, sl]),
            pk=pk_b))
    return in_maps


def _postprocess(results):
    out = np.empty(M + 1, np.float32)
    for c in range(N_CORES):
        out[c * MS:(c + 1) * MS] = results[c]["out"]
    out[M] = 0.0
    return out


def run_spmd(in_maps, trace=False):
    from concourse.bass_utils import run_bass_kernel_spmd
    nc = _build()
    return run_bass_kernel_spmd(nc, in_maps, list(range(N_CORES)), trace=trace)


def kernel(**inputs):
    in_maps = _prepare_maps(**inputs)
    res = run_spmd(in_maps, trace=False)
    return _postprocess(res.results)
